# revision 33
# baseline (speedup 1.0000x reference)
"""ArcFace loss kernel for 8 TRN2 NeuronCores (vocab/tensor-parallel).

reference:
    xn = normalize(x)               # [B, D]
    wn = normalize(weight)          # [C, D]
    logits = 64 * xn @ wn.T         # [B, C]
    loss = mean(CE(logits, label))

Strategy: shard classes C=100000 over 8 cores (12500 each, zero-padded to
12800 = 25*512). Host prepares normalized fp8(e4m3) operands scaled by G=8
(so device cosines are 64*cos), pre-packed in the exact SBUF tile layout so
every weight-group DMA is 8KB-contiguous per partition. Each core computes
its logit shard with TensorE fp8 DoubleRow matmuls (K=256/op) into fp32
PSUM.

ScalarE consumes each PSUM tile with one fused exp(l - SHIFT) +
row-accumulate activation (exact math; a fixed logsumexp shift is valid
since l <= 66). The small tail group is processed second so ScalarE builds
a lead over the PE during the DMA-limited start; all weight groups are
SBUF-resident, loaded once via both HWDGE queues with the first group's
k-halves split for the earliest possible compute start. Each core returns
its per-row partial sum-of-exp [128, 4]; the host gathers the 8 cores,
removes the exact zero-pad contribution, and finishes
loss = mean(log Z + SHIFT - 64*cos_label) with host-exact label cosines.
"""

import math
import numpy as np

import concourse.mybir as mybir
import concourse.tile as tile
from concourse import bacc
from concourse.bass_utils import run_bass_kernel_spmd

# Problem constants (hardcoded per harness contract).
B = 512
D = 512
C = 100000
S = 64.0
SHIFT = 20.0  # logsumexp shift; keeps Z ~1e-2 (HW Ln saturates below ~1e-19)
EPS = 1e-12
G = 8.0      # fp8 pre-scale on both operands: device cos' = G^2 * cos
NCORES = 8
CS = C // NCORES        # true classes per core = 12500
CHUNK = 512             # matmul moving free dim = one full PSUM bank
TAILC = 256             # tail chunk width (212 real + 44 pad cols)
CS_PAD = 24 * CHUNK + TAILC  # padded classes per core = 12544
GROUP = 4               # psum banks (512-col chunks) per group
NG_FULL = 6             # full groups of 4 chunks; + 1 tail group of 1 chunk
GCOLS = GROUP * CHUNK   # 2048 logit columns per full group
PB = 128                # partitions
KSUB = D // PB          # 4 contraction subtiles of 128
BBLK = B // PB          # 4 batch blocks
NG = NG_FULL + 1        # total groups per core
N_WARM = 26             # PE warm-up matmuls issued while the first DMAs land

AS = GCOLS              # activation width of a full-group tile

F32 = mybir.dt.float32
I32 = mybir.dt.int32
BF16 = mybir.dt.bfloat16
FP8 = mybir.dt.float8e4
NP_FP8 = mybir.dt.np(FP8)


def build_nc(ncores: int = NCORES):
    """Build the SPMD Bass graph."""
    nc = bacc.Bacc(
        "TRN2",
        target_bir_lowering=False,
        debug=False,
        num_devices=ncores,
    )

    # host-packed operands: per-partition-contiguous SBUF layouts
    wmain_ext = nc.dram_tensor(
        "wmain", [NG_FULL * PB, KSUB, GCOLS], FP8, kind="ExternalInput"
    )
    wtail_ext = nc.dram_tensor("wtail", [PB, KSUB, TAILC], FP8, kind="ExternalInput")
    xnt_ext = nc.dram_tensor("xnt", [PB, KSUB, B], FP8, kind="ExternalInput")
    zp_ext = nc.dram_tensor("zp", [PB, BBLK], F32, kind="ExternalOutput")

    with tile.TileContext(nc) as tc:
        with (
            tc.tile_pool(name="const", bufs=1) as cpool,
            tc.tile_pool(name="wpool", bufs=NG_FULL) as wpool,
            tc.tile_pool(name="dpool", bufs=3) as dpool,
        ):
            # x^T first on the SP HWDGE queue (it gates the first real
            # matmul); split by k-pair so the k2=0 matmuls start earlier
            xsb = cpool.tile([PB, KSUB, B], FP8)
            nc.sync.dma_start(out=xsb[:, 0:2, :], in_=xnt_ext[:, 0:2, :])
            nc.sync.dma_start(out=xsb[:, 2:4, :], in_=xnt_ext[:, 2:4, :])

            wts = []
            for g in range(NG_FULL):
                wt = wpool.tile([PB, KSUB, GCOLS], FP8, name="wt", tag="w")
                wts.append(wt)
            wtail = cpool.tile([PB, KSUB, TAILC], FP8)
            # early groups: k-halves split across both HWDGE queues so both
            # contribute bandwidth to each group the PE needs first
            nc.sync.dma_start(
                out=wts[0][:, 0:2, 0:1024], in_=wmain_ext[0:PB, 0:2, 0:1024]
            )
            nc.sync.dma_start(
                out=wts[0][:, 0:2, 1024:2048],
                in_=wmain_ext[0:PB, 0:2, 1024:2048],
            )
            nc.scalar.dma_start(
                out=wts[0][:, 2:4, :], in_=wmain_ext[0:PB, 2:4, :]
            )
            # the tail group runs second in compute order: its (small)
            # weights must arrive early
            nc.scalar.dma_start(out=wtail, in_=wtail_ext[:])
            for g in (1, 2):
                nc.sync.dma_start(
                    out=wts[g][:, 0:2, :],
                    in_=wmain_ext[g * PB : (g + 1) * PB, 0:2, :],
                )
                nc.scalar.dma_start(
                    out=wts[g][:, 2:4, :],
                    in_=wmain_ext[g * PB : (g + 1) * PB, 2:4, :],
                )
            nc.sync.dma_start(
                out=wts[3], in_=wmain_ext[3 * PB : 4 * PB, :, :]
            )
            nc.scalar.dma_start(
                out=wts[4], in_=wmain_ext[4 * PB : 5 * PB, :, :]
            )
            nc.sync.dma_start(
                out=wts[5], in_=wmain_ext[5 * PB : 6 * PB, :, :]
            )
            wts.append(wtail)

            # warm-up operand first so the PE can start immediately
            warm = cpool.tile([PB, 256], BF16)
            nc.vector.memset(warm, 0.0)

            # constants
            negs = cpool.tile([PB, 1], F32)
            nc.vector.memset(negs, -SHIFT)

            # per-row partial sums: one column per (b-block, group)
            pS = cpool.tile([PB, BBLK * NG], F32)

            # preload the Exp activation table off the critical path
            dumdum = cpool.tile([PB, 1], BF16)
            nc.scalar.activation(
                out=dumdum, in_=negs,
                func=mybir.ActivationFunctionType.Exp, bias=negs, scale=1.0,
            )

            with tc.tile_pool(name="psmain", bufs=2, space="PSUM") as pspool:
                # PE warm-up: dependency-free matmuls so the HAM clock gate is
                # released by the time the first weight tiles arrive.
                ones_bf = nc.const_aps.aps[(BF16, 1.0)]
                warm_ps = pspool.tile(
                    [PB, GROUP, CHUNK], F32, name="warm_ps", tag="ps",
                )
                for _ in range(N_WARM):
                    nc.tensor.matmul(
                        out=warm_ps[0:1, 0, :256], lhsT=ones_bf, rhs=warm,
                        start=True, stop=True,
                    )

                # tail-group tiles run half early (ScalarE builds a lead
                # during the DMA-limited start) and half last (the final
                # activation is then short, cutting the post-stream tail)
                tiles = [(0, bb) for bb in range(BBLK)]
                tiles += [(NG - 1, 0), (NG - 1, 1), (NG - 1, 2)]
                for g in range(1, NG - 1):
                    tiles += [(g, bb) for bb in range(BBLK)]
                tiles += [(NG - 1, 3)]
                for g, bb in tiles:
                    nsub = GROUP if g < NG_FULL else 1
                    ncols = nsub * CHUNK if g < NG_FULL else TAILC
                    acols = ncols
                    wt = wts[g]
                    if True:
                        ps = pspool.tile(
                            [PB, nsub, CHUNK], F32, name="ps", tag="ps",
                            padded_shape=[PB, GROUP, CHUNK],
                        )
                        cw = CHUNK if g < NG_FULL else TAILC
                        for k2 in range(KSUB // 2):
                            for sub in range(nsub):
                                nc.tensor.matmul(
                                    out=ps[:, sub : sub + 1, 0:cw],
                                    lhsT=xsb[
                                        :, 2 * k2 : 2 * k2 + 2,
                                        bb * PB : (bb + 1) * PB,
                                    ],
                                    rhs=wt[
                                        :, 2 * k2 : 2 * k2 + 2,
                                        sub * cw : (sub + 1) * cw,
                                    ],
                                    start=(k2 == 0),
                                    stop=(k2 == KSUB // 2 - 1),
                                    perf_mode=mybir.MatmulPerfMode.DoubleRow,
                                )
                        flat = ps.rearrange("p s c -> p (s c)")
                        col = bb * NG + g
                        # ScalarE: exp(l - SHIFT), row-accumulated
                        dump = dpool.tile(
                            [PB, acols], BF16, name="dump", tag="dump",
                            padded_shape=[PB, AS],
                        )
                        nc.scalar.activation(
                            out=dump,
                            in_=flat[:, 0:acols],
                            func=mybir.ActivationFunctionType.Exp,
                            bias=negs,
                            scale=1.0,
                            accum_out=pS[:, col : col + 1],
                        )

            # Z partial per row: [128, BBLK] -> output (host sums the 8 cores)
            zp = cpool.tile([PB, BBLK], F32)
            nc.vector.tensor_reduce(
                zp,
                pS.rearrange("p (b g) -> p b g", b=BBLK),
                axis=mybir.AxisListType.X,
                op=mybir.AluOpType.add,
            )
            nc.sync.dma_start(out=zp_ext[:], in_=zp)

    nc.finalize()
    return nc


def prepare_inputs(x, weight, label, ncores: int = NCORES):
    """Host-side prep: normalize, G-scale, cast fp8, pack to SBUF layouts.

    Returns (in_maps, lc2) where lc2[p, j] = SHIFT - S*cos(x_b, w_label_b)
    for b = j*128 + p."""
    x = np.asarray(x, dtype=np.float32)
    weight = np.asarray(weight, dtype=np.float32)
    label = np.asarray(label).astype(np.int64)

    xn = x / np.maximum(
        np.sqrt(np.einsum("bd,bd->b", x, x, dtype=np.float64))[:, None], EPS
    ).astype(np.float32)
    wnorm = np.sqrt(np.einsum("cd,cd->c", weight, weight, dtype=np.float64))
    wn = weight / np.maximum(wnorm[:, None], EPS).astype(np.float32)

    # label cosine computed on host in f64 (exact vs fp32 reference)
    wl = wn[label]  # [B, D]
    label_cos = np.einsum("bd,bd->b", xn.astype(np.float64), wl.astype(np.float64))
    lc2 = (SHIFT - S * label_cos).astype(np.float64)  # [B]
    lc2_pj = np.ascontiguousarray(lc2.reshape(BBLK, PB).T)  # [128, BBLK]

    x8 = (G * xn).astype(NP_FP8)          # [B, D]
    w8 = (G * wn).astype(NP_FP8)          # [C, D]
    # xnt[p, ks, b] = x8[b, ks*128 + p]
    xp = np.ascontiguousarray(x8.reshape(B, KSUB, PB).transpose(2, 1, 0))

    in_maps = []
    for i in range(ncores):
        wp = np.zeros((CS_PAD, D), dtype=NP_FP8)
        wp[:CS] = w8[i * CS : (i + 1) * CS]
        # wmain[g*128+p, ks, col] = wp[g*2048 + col, ks*128 + p]
        main = (
            wp[: NG_FULL * GCOLS]
            .reshape(NG_FULL, GCOLS, KSUB, PB)
            .transpose(0, 3, 2, 1)
            .reshape(NG_FULL * PB, KSUB, GCOLS)
        )
        tail = wp[NG_FULL * GCOLS :].reshape(TAILC, KSUB, PB).transpose(2, 1, 0)
        in_maps.append(
            {
                "wmain": np.ascontiguousarray(main),
                "wtail": np.ascontiguousarray(tail),
                "xnt": xp,
            }
        )
    return in_maps, lc2_pj


_NC_CACHE = {}


def _get_nc():
    if "nc" not in _NC_CACHE:
        _NC_CACHE["nc"] = build_nc()
    return _NC_CACHE["nc"]


def _install_ntff_hook():
    """The agent image's antenv lacks axon_hooks; shim it so trace=True can
    capture NTFF profiles via the ctypes hook in trn_agent_boot."""
    import sys
    import types

    try:
        from antenv.axon_hooks import get_axon_ntff_profile_hook  # noqa: F401
        return
    except ImportError:
        pass
    mod = types.ModuleType("antenv.axon_hooks")
    _state = {"hook": None}
    mod.set_axon_ntff_profile_hook = lambda h: _state.__setitem__("hook", h)
    mod.get_axon_ntff_profile_hook = lambda: _state["hook"]
    sys.modules["antenv.axon_hooks"] = mod
    import antenv

    antenv.axon_hooks = mod
    from trn_agent_boot.trn_boot import _ntff_profile_via_ctypes

    mod.set_axon_ntff_profile_hook(
        _ntff_profile_via_ctypes("/opt/axon/libaxon_pjrt.so")
    )
    # keep trace artifacts local (no external upload from this sandbox)
    import concourse.bass_utils as bu

    bu.upload_artifacts = lambda tmpdir: tmpdir


def finish_loss(results, lc2_pj):
    """Host epilogue: sum the 8 per-core partials, remove the exact
    zero-pad contribution, log, add label term, mean."""
    Z = np.zeros((PB, BBLK), dtype=np.float64)
    for r in results:
        Z += r["zp"].astype(np.float64)
    # pads: tail-tile cols 212..511 are zeros; the S-path covers [0, AS_T)
    # and the P-path [AS_T, 512)
    n_pad = CS_PAD - CS                      # 44, all exact exp(-SHIFT)
    Z -= NCORES * n_pad * math.exp(-SHIFT)
    loss = float((np.log(Z) + lc2_pj).mean())
    return np.float32(loss)


def run(x, weight, label, trace=False):
    """Returns (loss_scalar, BassKernelResults)."""
    if trace:
        _install_ntff_hook()
    nc = _get_nc()
    in_maps, lc2_pj = prepare_inputs(x, weight, label)
    res = run_bass_kernel_spmd(
        nc, in_maps, core_ids=list(range(NCORES)), trace=trace
    )
    loss = finish_loss(res.results, lc2_pj)
    return loss, res


def kernel(x, weight, label, batch=None, **_ignored):
    loss, _ = run(x, weight, label, trace=False)
    return np.asarray(loss, dtype=np.float32)


# revision 34
# speedup vs baseline: 1.0277x; 1.0277x over previous
"""ArcFace loss kernel for 8 TRN2 NeuronCores (vocab/tensor-parallel).

reference:
    xn = normalize(x)               # [B, D]
    wn = normalize(weight)          # [C, D]
    logits = 64 * xn @ wn.T         # [B, C]
    loss = mean(CE(logits, label))

Strategy: shard classes C=100000 over 8 cores (12500 each, zero-padded to
12800 = 25*512). Host prepares normalized fp8(e4m3) operands scaled by G=8
(so device cosines are 64*cos), pre-packed in the exact SBUF tile layout so
every weight-group DMA is 8KB-contiguous per partition. Each core computes
its logit shard with TensorE fp8 DoubleRow matmuls (K=256/op) into fp32
PSUM.

ScalarE consumes each PSUM tile with one fused exp(l - SHIFT) +
row-accumulate activation (exact math; a fixed logsumexp shift is valid
since l <= 66). The small tail group is processed second so ScalarE builds
a lead over the PE during the DMA-limited start; all weight groups are
SBUF-resident, loaded once via both HWDGE queues with the first group's
k-halves split for the earliest possible compute start. Each core returns
its per-row partial sum-of-exp [128, 4]; the host gathers the 8 cores,
removes the exact zero-pad contribution, and finishes
loss = mean(log Z + SHIFT - 64*cos_label) with host-exact label cosines.
"""

import math
import numpy as np

import concourse.mybir as mybir
import concourse.tile as tile
from concourse import bacc
from concourse.bass_utils import run_bass_kernel_spmd

# Problem constants (hardcoded per harness contract).
B = 512
D = 512
C = 100000
S = 64.0
SHIFT = 20.0  # logsumexp shift; keeps Z ~1e-2 (HW Ln saturates below ~1e-19)
EPS = 1e-12
G = 8.0      # fp8 pre-scale on both operands: device cos' = G^2 * cos
NCORES = 8
CS = C // NCORES        # true classes per core = 12500
CHUNK = 512             # matmul moving free dim = one full PSUM bank
TAILC = 256             # tail chunk width (212 real + 44 pad cols)
CS_PAD = 24 * CHUNK + TAILC  # padded classes per core = 12544
GROUP = 4               # psum banks (512-col chunks) per group
NG_FULL = 6             # full groups of 4 chunks; + 1 tail group of 1 chunk
GCOLS = GROUP * CHUNK   # 2048 logit columns per full group
PB = 128                # partitions
KSUB = D // PB          # 4 contraction subtiles of 128
BBLK = B // PB          # 4 batch blocks
NG = NG_FULL + 1        # total groups per core
N_WARM = 26             # PE warm-up matmuls issued while the first DMAs land

AS = GCOLS              # activation width of a full-group tile

F32 = mybir.dt.float32
I32 = mybir.dt.int32
BF16 = mybir.dt.bfloat16
FP8 = mybir.dt.float8e4
NP_FP8 = mybir.dt.np(FP8)


def build_nc(ncores: int = NCORES):
    """Build the SPMD Bass graph."""
    nc = bacc.Bacc(
        "TRN2",
        target_bir_lowering=False,
        debug=False,
        num_devices=ncores,
    )

    # host-packed operands: per-partition-contiguous SBUF layouts
    wmain_ext = nc.dram_tensor(
        "wmain", [NG_FULL * PB, KSUB, GCOLS], FP8, kind="ExternalInput"
    )
    wtail_ext = nc.dram_tensor("wtail", [PB, KSUB, TAILC], FP8, kind="ExternalInput")
    xnt_ext = nc.dram_tensor("xnt", [PB, KSUB, B], FP8, kind="ExternalInput")
    zp_ext = nc.dram_tensor("zp", [PB, BBLK], F32, kind="ExternalOutput")

    with tile.TileContext(nc) as tc:
        with (
            tc.tile_pool(name="const", bufs=1) as cpool,
            tc.tile_pool(name="wpool", bufs=NG_FULL) as wpool,
            tc.tile_pool(name="dpool", bufs=3) as dpool,
        ):
            # x^T first on the SP HWDGE queue; one DMA = 128 contiguous
            # 2KB descriptors (k-splitting doubles the descriptor count and
            # delays the weight loads behind it at the HWDGE descriptor rate)
            xsb = cpool.tile([PB, KSUB, B], FP8)
            nc.sync.dma_start(out=xsb, in_=xnt_ext[:])

            wts = []
            for g in range(NG_FULL):
                wt = wpool.tile([PB, KSUB, GCOLS], FP8, name="wt", tag="w")
                wts.append(wt)
            wtail = cpool.tile([PB, KSUB, TAILC], FP8)
            # early groups: k-halves split across both HWDGE queues so both
            # contribute bandwidth to each group the PE needs first
            nc.sync.dma_start(
                out=wts[0][:, 0:2, :], in_=wmain_ext[0:PB, 0:2, :]
            )
            nc.scalar.dma_start(
                out=wts[0][:, 2:4, :], in_=wmain_ext[0:PB, 2:4, :]
            )
            # the tail group runs second in compute order: its (small)
            # weights must arrive early
            nc.scalar.dma_start(out=wtail, in_=wtail_ext[:])
            for g in (1, 2):
                nc.sync.dma_start(
                    out=wts[g][:, 0:2, :],
                    in_=wmain_ext[g * PB : (g + 1) * PB, 0:2, :],
                )
                nc.scalar.dma_start(
                    out=wts[g][:, 2:4, :],
                    in_=wmain_ext[g * PB : (g + 1) * PB, 2:4, :],
                )
            nc.sync.dma_start(
                out=wts[3], in_=wmain_ext[3 * PB : 4 * PB, :, :]
            )
            nc.scalar.dma_start(
                out=wts[4], in_=wmain_ext[4 * PB : 5 * PB, :, :]
            )
            nc.sync.dma_start(
                out=wts[5], in_=wmain_ext[5 * PB : 6 * PB, :, :]
            )
            wts.append(wtail)

            # warm-up operand first so the PE can start immediately
            warm = cpool.tile([PB, 256], BF16)
            nc.vector.memset(warm, 0.0)

            # constants
            negs = cpool.tile([PB, 1], F32)
            nc.vector.memset(negs, -SHIFT)

            # per-row partial sums: one column per (b-block, group)
            pS = cpool.tile([PB, BBLK * NG], F32)

            # preload the Exp activation table off the critical path
            dumdum = cpool.tile([PB, 1], BF16)
            nc.scalar.activation(
                out=dumdum, in_=negs,
                func=mybir.ActivationFunctionType.Exp, bias=negs, scale=1.0,
            )

            with tc.tile_pool(name="psmain", bufs=2, space="PSUM") as pspool:
                # PE warm-up: dependency-free matmuls so the HAM clock gate is
                # released by the time the first weight tiles arrive.
                ones_bf = nc.const_aps.aps[(BF16, 1.0)]
                warm_ps = pspool.tile(
                    [PB, GROUP, CHUNK], F32, name="warm_ps", tag="ps",
                )
                for _ in range(N_WARM):
                    nc.tensor.matmul(
                        out=warm_ps[0:1, 0, :256], lhsT=ones_bf, rhs=warm,
                        start=True, stop=True,
                    )

                # tail-group tiles run half early (ScalarE builds a lead
                # during the DMA-limited start) and half last (the final
                # activation is then short, cutting the post-stream tail)
                tiles = [(0, bb) for bb in range(BBLK)]
                tiles += [(NG - 1, 0), (NG - 1, 1), (NG - 1, 2)]
                for g in range(1, NG - 1):
                    tiles += [(g, bb) for bb in range(BBLK)]
                tiles += [(NG - 1, 3)]
                for g, bb in tiles:
                    nsub = GROUP if g < NG_FULL else 1
                    ncols = nsub * CHUNK if g < NG_FULL else TAILC
                    acols = ncols
                    wt = wts[g]
                    if True:
                        ps = pspool.tile(
                            [PB, nsub, CHUNK], F32, name="ps", tag="ps",
                            padded_shape=[PB, GROUP, CHUNK],
                        )
                        cw = CHUNK if g < NG_FULL else TAILC
                        for k2 in range(KSUB // 2):
                            for sub in range(nsub):
                                nc.tensor.matmul(
                                    out=ps[:, sub : sub + 1, 0:cw],
                                    lhsT=xsb[
                                        :, 2 * k2 : 2 * k2 + 2,
                                        bb * PB : (bb + 1) * PB,
                                    ],
                                    rhs=wt[
                                        :, 2 * k2 : 2 * k2 + 2,
                                        sub * cw : (sub + 1) * cw,
                                    ],
                                    start=(k2 == 0),
                                    stop=(k2 == KSUB // 2 - 1),
                                    perf_mode=mybir.MatmulPerfMode.DoubleRow,
                                )
                        flat = ps.rearrange("p s c -> p (s c)")
                        col = bb * NG + g
                        # ScalarE: exp(l - SHIFT), row-accumulated
                        dump = dpool.tile(
                            [PB, acols], BF16, name="dump", tag="dump",
                            padded_shape=[PB, AS],
                        )
                        nc.scalar.activation(
                            out=dump,
                            in_=flat[:, 0:acols],
                            func=mybir.ActivationFunctionType.Exp,
                            bias=negs,
                            scale=1.0,
                            accum_out=pS[:, col : col + 1],
                        )

            # Z partial per row: [128, BBLK] -> output (host sums the 8 cores)
            zp = cpool.tile([PB, BBLK], F32)
            nc.vector.tensor_reduce(
                zp,
                pS.rearrange("p (b g) -> p b g", b=BBLK),
                axis=mybir.AxisListType.X,
                op=mybir.AluOpType.add,
            )
            nc.sync.dma_start(out=zp_ext[:], in_=zp)

    nc.finalize()
    return nc


def prepare_inputs(x, weight, label, ncores: int = NCORES):
    """Host-side prep: normalize, G-scale, cast fp8, pack to SBUF layouts.

    Returns (in_maps, lc2) where lc2[p, j] = SHIFT - S*cos(x_b, w_label_b)
    for b = j*128 + p."""
    x = np.asarray(x, dtype=np.float32)
    weight = np.asarray(weight, dtype=np.float32)
    label = np.asarray(label).astype(np.int64)

    xn = x / np.maximum(
        np.sqrt(np.einsum("bd,bd->b", x, x, dtype=np.float64))[:, None], EPS
    ).astype(np.float32)
    wnorm = np.sqrt(np.einsum("cd,cd->c", weight, weight, dtype=np.float64))
    wn = weight / np.maximum(wnorm[:, None], EPS).astype(np.float32)

    # label cosine computed on host in f64 (exact vs fp32 reference)
    wl = wn[label]  # [B, D]
    label_cos = np.einsum("bd,bd->b", xn.astype(np.float64), wl.astype(np.float64))
    lc2 = (SHIFT - S * label_cos).astype(np.float64)  # [B]
    lc2_pj = np.ascontiguousarray(lc2.reshape(BBLK, PB).T)  # [128, BBLK]

    x8 = (G * xn).astype(NP_FP8)          # [B, D]
    w8 = (G * wn).astype(NP_FP8)          # [C, D]
    # xnt[p, ks, b] = x8[b, ks*128 + p]
    xp = np.ascontiguousarray(x8.reshape(B, KSUB, PB).transpose(2, 1, 0))

    in_maps = []
    for i in range(ncores):
        wp = np.zeros((CS_PAD, D), dtype=NP_FP8)
        wp[:CS] = w8[i * CS : (i + 1) * CS]
        # wmain[g*128+p, ks, col] = wp[g*2048 + col, ks*128 + p]
        main = (
            wp[: NG_FULL * GCOLS]
            .reshape(NG_FULL, GCOLS, KSUB, PB)
            .transpose(0, 3, 2, 1)
            .reshape(NG_FULL * PB, KSUB, GCOLS)
        )
        tail = wp[NG_FULL * GCOLS :].reshape(TAILC, KSUB, PB).transpose(2, 1, 0)
        in_maps.append(
            {
                "wmain": np.ascontiguousarray(main),
                "wtail": np.ascontiguousarray(tail),
                "xnt": xp,
            }
        )
    return in_maps, lc2_pj


_NC_CACHE = {}


def _get_nc():
    if "nc" not in _NC_CACHE:
        _NC_CACHE["nc"] = build_nc()
    return _NC_CACHE["nc"]


def _install_ntff_hook():
    """The agent image's antenv lacks axon_hooks; shim it so trace=True can
    capture NTFF profiles via the ctypes hook in trn_agent_boot."""
    import sys
    import types

    try:
        from antenv.axon_hooks import get_axon_ntff_profile_hook  # noqa: F401
        return
    except ImportError:
        pass
    mod = types.ModuleType("antenv.axon_hooks")
    _state = {"hook": None}
    mod.set_axon_ntff_profile_hook = lambda h: _state.__setitem__("hook", h)
    mod.get_axon_ntff_profile_hook = lambda: _state["hook"]
    sys.modules["antenv.axon_hooks"] = mod
    import antenv

    antenv.axon_hooks = mod
    from trn_agent_boot.trn_boot import _ntff_profile_via_ctypes

    mod.set_axon_ntff_profile_hook(
        _ntff_profile_via_ctypes("/opt/axon/libaxon_pjrt.so")
    )
    # keep trace artifacts local (no external upload from this sandbox)
    import concourse.bass_utils as bu

    bu.upload_artifacts = lambda tmpdir: tmpdir


def finish_loss(results, lc2_pj):
    """Host epilogue: sum the 8 per-core partials, remove the exact
    zero-pad contribution, log, add label term, mean."""
    Z = np.zeros((PB, BBLK), dtype=np.float64)
    for r in results:
        Z += r["zp"].astype(np.float64)
    # pads: tail-tile cols 212..511 are zeros; the S-path covers [0, AS_T)
    # and the P-path [AS_T, 512)
    n_pad = CS_PAD - CS                      # 44, all exact exp(-SHIFT)
    Z -= NCORES * n_pad * math.exp(-SHIFT)
    loss = float((np.log(Z) + lc2_pj).mean())
    return np.float32(loss)


def run(x, weight, label, trace=False):
    """Returns (loss_scalar, BassKernelResults)."""
    if trace:
        _install_ntff_hook()
    nc = _get_nc()
    in_maps, lc2_pj = prepare_inputs(x, weight, label)
    res = run_bass_kernel_spmd(
        nc, in_maps, core_ids=list(range(NCORES)), trace=trace
    )
    loss = finish_loss(res.results, lc2_pj)
    return loss, res


def kernel(x, weight, label, batch=None, **_ignored):
    loss, _ = run(x, weight, label, trace=False)
    return np.asarray(loss, dtype=np.float32)


# revision 35
# speedup vs baseline: 1.0482x; 1.0199x over previous
"""ArcFace loss kernel for 8 TRN2 NeuronCores (vocab/tensor-parallel).

reference:
    xn = normalize(x)               # [B, D]
    wn = normalize(weight)          # [C, D]
    logits = 64 * xn @ wn.T         # [B, C]
    loss = mean(CE(logits, label))

Strategy: shard classes C=100000 over 8 cores (12500 each, zero-padded to
12800 = 25*512). Host prepares normalized fp8(e4m3) operands scaled by G=8
(so device cosines are 64*cos), pre-packed in the exact SBUF tile layout so
every weight-group DMA is 8KB-contiguous per partition. Each core computes
its logit shard with TensorE fp8 DoubleRow matmuls (K=256/op) into fp32
PSUM.

ScalarE consumes each PSUM tile with one fused exp(l - SHIFT) +
row-accumulate activation (exact math; a fixed logsumexp shift is valid
since l <= 66). The small tail group is processed second so ScalarE builds
a lead over the PE during the DMA-limited start; all weight groups are
SBUF-resident, loaded once via both HWDGE queues with the first group's
k-halves split for the earliest possible compute start. Each core returns
its per-row partial sum-of-exp [128, 4]; the host gathers the 8 cores,
removes the exact zero-pad contribution, and finishes
loss = mean(log Z + SHIFT - 64*cos_label) with host-exact label cosines.
"""

import math
import numpy as np

import concourse.mybir as mybir
import concourse.tile as tile
from concourse import bacc
from concourse.bass_utils import run_bass_kernel_spmd

# Problem constants (hardcoded per harness contract).
B = 512
D = 512
C = 100000
S = 64.0
SHIFT = 20.0  # logsumexp shift; keeps Z ~1e-2 (HW Ln saturates below ~1e-19)
EPS = 1e-12
G = 8.0      # fp8 pre-scale on both operands: device cos' = G^2 * cos
NCORES = 8
CS = C // NCORES        # true classes per core = 12500
CHUNK = 512             # matmul moving free dim = one full PSUM bank
TAILC = 256             # tail chunk width (212 real + 44 pad cols)
CS_PAD = 24 * CHUNK + TAILC  # padded classes per core = 12544
GROUP = 4               # psum banks (512-col chunks) per group
NG_FULL = 6             # full groups of 4 chunks; + 1 tail group of 1 chunk
GCOLS = GROUP * CHUNK   # 2048 logit columns per full group
PB = 128                # partitions
KSUB = D // PB          # 4 contraction subtiles of 128
BBLK = B // PB          # 4 batch blocks
NG = NG_FULL + 1        # total groups per core
N_WARM = 32             # PE warm-up matmuls issued while the first DMAs land

AS = GCOLS              # activation width of a full-group tile

F32 = mybir.dt.float32
I32 = mybir.dt.int32
BF16 = mybir.dt.bfloat16
FP8 = mybir.dt.float8e4
NP_FP8 = mybir.dt.np(FP8)


def build_nc(ncores: int = NCORES):
    """Build the SPMD Bass graph."""
    nc = bacc.Bacc(
        "TRN2",
        target_bir_lowering=False,
        debug=False,
        num_devices=ncores,
    )

    # host-packed operands: per-partition-contiguous SBUF layouts
    wmain_ext = nc.dram_tensor(
        "wmain", [NG_FULL * PB, KSUB, GCOLS], FP8, kind="ExternalInput"
    )
    wtail_ext = nc.dram_tensor("wtail", [PB, KSUB, TAILC], FP8, kind="ExternalInput")
    xnt_ext = nc.dram_tensor("xnt", [PB, KSUB, B], FP8, kind="ExternalInput")
    zp_ext = nc.dram_tensor("zp", [PB, BBLK], F32, kind="ExternalOutput")

    with tile.TileContext(nc) as tc:
        with (
            tc.tile_pool(name="const", bufs=1) as cpool,
            tc.tile_pool(name="wpool", bufs=NG_FULL) as wpool,
            tc.tile_pool(name="dpool", bufs=3) as dpool,
        ):
            # x^T first on the SP HWDGE queue; one DMA = 128 contiguous
            # 2KB descriptors (k-splitting doubles the descriptor count and
            # delays the weight loads behind it at the HWDGE descriptor rate)
            xsb = cpool.tile([PB, KSUB, B], FP8)
            nc.sync.dma_start(out=xsb, in_=xnt_ext[:])

            wts = []
            for g in range(NG_FULL):
                wt = wpool.tile([PB, KSUB, GCOLS], FP8, name="wt", tag="w")
                wts.append(wt)
            wtail = cpool.tile([PB, KSUB, TAILC], FP8)
            # early groups: k-halves split across both HWDGE queues so both
            # contribute bandwidth to each group the PE needs first
            nc.sync.dma_start(
                out=wts[0][:, 0:2, :], in_=wmain_ext[0:PB, 0:2, :]
            )
            nc.scalar.dma_start(
                out=wts[0][:, 2:4, :], in_=wmain_ext[0:PB, 2:4, :]
            )
            # the tail group runs second in compute order: its (small)
            # weights must arrive early
            nc.scalar.dma_start(out=wtail, in_=wtail_ext[:])
            for g in (1, 2):
                nc.sync.dma_start(
                    out=wts[g][:, 0:2, :],
                    in_=wmain_ext[g * PB : (g + 1) * PB, 0:2, :],
                )
                nc.scalar.dma_start(
                    out=wts[g][:, 2:4, :],
                    in_=wmain_ext[g * PB : (g + 1) * PB, 2:4, :],
                )
            nc.sync.dma_start(
                out=wts[3], in_=wmain_ext[3 * PB : 4 * PB, :, :]
            )
            nc.scalar.dma_start(
                out=wts[4], in_=wmain_ext[4 * PB : 5 * PB, :, :]
            )
            nc.sync.dma_start(
                out=wts[5], in_=wmain_ext[5 * PB : 6 * PB, :, :]
            )
            wts.append(wtail)

            # warm-up operand first so the PE can start immediately
            warm = cpool.tile([PB, 256], BF16)
            nc.vector.memset(warm, 0.0)

            # constants
            negs = cpool.tile([PB, 1], F32)
            nc.vector.memset(negs, -SHIFT)

            # per-row partial sums: one column per (b-block, group)
            pS = cpool.tile([PB, BBLK * NG], F32)

            # preload the Exp activation table off the critical path
            dumdum = cpool.tile([PB, 1], BF16)
            nc.scalar.activation(
                out=dumdum, in_=negs,
                func=mybir.ActivationFunctionType.Exp, bias=negs, scale=1.0,
            )

            with tc.tile_pool(name="psmain", bufs=2, space="PSUM") as pspool:
                # PE warm-up: dependency-free matmuls so the HAM clock gate is
                # released by the time the first weight tiles arrive.
                ones_bf = nc.const_aps.aps[(BF16, 1.0)]
                warm_ps = pspool.tile(
                    [PB, GROUP, CHUNK], F32, name="warm_ps", tag="ps",
                )
                for _ in range(N_WARM):
                    nc.tensor.matmul(
                        out=warm_ps[0:1, 0, :256], lhsT=ones_bf, rhs=warm,
                        start=True, stop=True,
                    )

                # tail-group tiles run half early (ScalarE builds a lead
                # during the DMA-limited start) and half last (the final
                # activation is then short, cutting the post-stream tail)
                tiles = [(0, bb) for bb in range(BBLK)]
                tiles += [(NG - 1, 0), (NG - 1, 1), (NG - 1, 2)]
                for g in range(1, NG - 1):
                    tiles += [(g, bb) for bb in range(BBLK)]
                tiles += [(NG - 1, 3)]
                for g, bb in tiles:
                    nsub = GROUP if g < NG_FULL else 1
                    ncols = nsub * CHUNK if g < NG_FULL else TAILC
                    acols = ncols
                    wt = wts[g]
                    if True:
                        ps = pspool.tile(
                            [PB, nsub, CHUNK], F32, name="ps", tag="ps",
                            padded_shape=[PB, GROUP, CHUNK],
                        )
                        cw = CHUNK if g < NG_FULL else TAILC
                        for k2 in range(KSUB // 2):
                            for sub in range(nsub):
                                nc.tensor.matmul(
                                    out=ps[:, sub : sub + 1, 0:cw],
                                    lhsT=xsb[
                                        :, 2 * k2 : 2 * k2 + 2,
                                        bb * PB : (bb + 1) * PB,
                                    ],
                                    rhs=wt[
                                        :, 2 * k2 : 2 * k2 + 2,
                                        sub * cw : (sub + 1) * cw,
                                    ],
                                    start=(k2 == 0),
                                    stop=(k2 == KSUB // 2 - 1),
                                    perf_mode=mybir.MatmulPerfMode.DoubleRow,
                                )
                        flat = ps.rearrange("p s c -> p (s c)")
                        col = bb * NG + g
                        # ScalarE: exp(l - SHIFT), row-accumulated
                        dump = dpool.tile(
                            [PB, acols], BF16, name="dump", tag="dump",
                            padded_shape=[PB, AS],
                        )
                        nc.scalar.activation(
                            out=dump,
                            in_=flat[:, 0:acols],
                            func=mybir.ActivationFunctionType.Exp,
                            bias=negs,
                            scale=1.0,
                            accum_out=pS[:, col : col + 1],
                        )

            # Z partial per row: [128, BBLK] -> output (host sums the 8 cores)
            zp = cpool.tile([PB, BBLK], F32)
            nc.vector.tensor_reduce(
                zp,
                pS.rearrange("p (b g) -> p b g", b=BBLK),
                axis=mybir.AxisListType.X,
                op=mybir.AluOpType.add,
            )
            nc.sync.dma_start(out=zp_ext[:], in_=zp)

    nc.finalize()
    return nc


def prepare_inputs(x, weight, label, ncores: int = NCORES):
    """Host-side prep: normalize, G-scale, cast fp8, pack to SBUF layouts.

    Returns (in_maps, lc2) where lc2[p, j] = SHIFT - S*cos(x_b, w_label_b)
    for b = j*128 + p."""
    x = np.asarray(x, dtype=np.float32)
    weight = np.asarray(weight, dtype=np.float32)
    label = np.asarray(label).astype(np.int64)

    xn = x / np.maximum(
        np.sqrt(np.einsum("bd,bd->b", x, x, dtype=np.float64))[:, None], EPS
    ).astype(np.float32)
    wnorm = np.sqrt(np.einsum("cd,cd->c", weight, weight, dtype=np.float64))
    wn = weight / np.maximum(wnorm[:, None], EPS).astype(np.float32)

    # label cosine computed on host in f64 (exact vs fp32 reference)
    wl = wn[label]  # [B, D]
    label_cos = np.einsum("bd,bd->b", xn.astype(np.float64), wl.astype(np.float64))
    lc2 = (SHIFT - S * label_cos).astype(np.float64)  # [B]
    lc2_pj = np.ascontiguousarray(lc2.reshape(BBLK, PB).T)  # [128, BBLK]

    x8 = (G * xn).astype(NP_FP8)          # [B, D]
    w8 = (G * wn).astype(NP_FP8)          # [C, D]
    # xnt[p, ks, b] = x8[b, ks*128 + p]
    xp = np.ascontiguousarray(x8.reshape(B, KSUB, PB).transpose(2, 1, 0))

    in_maps = []
    for i in range(ncores):
        wp = np.zeros((CS_PAD, D), dtype=NP_FP8)
        wp[:CS] = w8[i * CS : (i + 1) * CS]
        # wmain[g*128+p, ks, col] = wp[g*2048 + col, ks*128 + p]
        main = (
            wp[: NG_FULL * GCOLS]
            .reshape(NG_FULL, GCOLS, KSUB, PB)
            .transpose(0, 3, 2, 1)
            .reshape(NG_FULL * PB, KSUB, GCOLS)
        )
        tail = wp[NG_FULL * GCOLS :].reshape(TAILC, KSUB, PB).transpose(2, 1, 0)
        in_maps.append(
            {
                "wmain": np.ascontiguousarray(main),
                "wtail": np.ascontiguousarray(tail),
                "xnt": xp,
            }
        )
    return in_maps, lc2_pj


_NC_CACHE = {}


def _get_nc():
    if "nc" not in _NC_CACHE:
        _NC_CACHE["nc"] = build_nc()
    return _NC_CACHE["nc"]


def _install_ntff_hook():
    """The agent image's antenv lacks axon_hooks; shim it so trace=True can
    capture NTFF profiles via the ctypes hook in trn_agent_boot."""
    import sys
    import types

    try:
        from antenv.axon_hooks import get_axon_ntff_profile_hook  # noqa: F401
        return
    except ImportError:
        pass
    mod = types.ModuleType("antenv.axon_hooks")
    _state = {"hook": None}
    mod.set_axon_ntff_profile_hook = lambda h: _state.__setitem__("hook", h)
    mod.get_axon_ntff_profile_hook = lambda: _state["hook"]
    sys.modules["antenv.axon_hooks"] = mod
    import antenv

    antenv.axon_hooks = mod
    from trn_agent_boot.trn_boot import _ntff_profile_via_ctypes

    mod.set_axon_ntff_profile_hook(
        _ntff_profile_via_ctypes("/opt/axon/libaxon_pjrt.so")
    )
    # keep trace artifacts local (no external upload from this sandbox)
    import concourse.bass_utils as bu

    bu.upload_artifacts = lambda tmpdir: tmpdir


def finish_loss(results, lc2_pj):
    """Host epilogue: sum the 8 per-core partials, remove the exact
    zero-pad contribution, log, add label term, mean."""
    Z = np.zeros((PB, BBLK), dtype=np.float64)
    for r in results:
        Z += r["zp"].astype(np.float64)
    # pads: tail-tile cols 212..511 are zeros; the S-path covers [0, AS_T)
    # and the P-path [AS_T, 512)
    n_pad = CS_PAD - CS                      # 44, all exact exp(-SHIFT)
    Z -= NCORES * n_pad * math.exp(-SHIFT)
    loss = float((np.log(Z) + lc2_pj).mean())
    return np.float32(loss)


def run(x, weight, label, trace=False):
    """Returns (loss_scalar, BassKernelResults)."""
    if trace:
        _install_ntff_hook()
    nc = _get_nc()
    in_maps, lc2_pj = prepare_inputs(x, weight, label)
    res = run_bass_kernel_spmd(
        nc, in_maps, core_ids=list(range(NCORES)), trace=trace
    )
    loss = finish_loss(res.results, lc2_pj)
    return loss, res


def kernel(x, weight, label, batch=None, **_ignored):
    loss, _ = run(x, weight, label, trace=False)
    return np.asarray(loss, dtype=np.float32)
